# revision 22
# baseline (speedup 1.0000x reference)
"""Trainium2 Bass kernel for ContinualLoraMoeOneGateInjectedLinear.

Computation (see reference):
    route  = lora_route[task_id-1]            (or sum over tasks if task_id > 5)
    a      = x @ route                        [B,S,P]
    gate   = 2*mean(softmax(a, -1), S) - 1    [B,P]
    tid    = min(task_id, 5)
    delta  = sum_p gate[b,p] * (x @ down_p) @ up_p   (p < tid)
    y      = x @ linear_w.T + delta

Sharding: 8 cores = 4 batches x 2 output-halves.  Core k handles batch
k//2 and output columns [512*(k%2), 512*(k%2+1)).  Each core sees all
tokens of its batch, so the gate is computed locally - no collectives.

Device kernel (per core), all heavy matmuls in float32r:
  1. zaT[45, 4096] = [down|route].T @ x   (one pass over x)
  2. transpose routing logits to [4096, 5], softmax, token-sum -> gate[5]
  3. y_base[4096, 512] = x @ Wt_half      (streams concurrently)
  4. y = y_base + zT.T @ (gate-scaled up) (added during PSUM drain)
"""

import sys

if "/opt/trn_rl_repo" not in sys.path:
    sys.path.insert(0, "/opt/trn_rl_repo")

from contextlib import ExitStack

import numpy as np

import concourse.bass as bass
import concourse.mybir as mybir
import concourse.tile as tile
from concourse import bacc
from concourse.bass_utils import run_bass_kernel_spmd

F32 = mybir.dt.float32
F32R = mybir.dt.float32r
F16 = mybir.dt.float16

NUM_TASKS = 5
B, S, IN, OUT, P, R = 4, 4096, 1024, 1024, 5, 8
RT = P * R  # 40 total low-rank dims
ZA = 72  # fused [down|route] matmul rows: 0:40 down, 64:69 route (32-aligned)
RB = 64  # partition base of the route rows
OH = OUT // 2  # 512, per-core output half
NI = IN // 128  # 8 i-tiles
NC = S // 512  # 8 token chunks of 512
NG = S // 128  # 32 token tiles of 128


def build_kernel():
    """Build the per-core Bacc module (identical program on all 8 cores)."""
    nc = bacc.Bacc("TRN2", target_bir_lowering=False, debug=False, num_devices=8)

    xt_d = nc.dram_tensor("xt", [128, NC, NI * 512], F16, kind="ExternalInput").ap()
    wt_d = nc.dram_tensor("wt", [128, NI * OH], F16, kind="ExternalInput").ap()
    rd_d = nc.dram_tensor("rd", [128, NI * ZA], F16, kind="ExternalInput").ap()
    up_d = nc.dram_tensor("up", [RT, OH], F32, kind="ExternalInput").ap()
    eye_d = nc.dram_tensor("eye5", [P, P], F32, kind="ExternalInput").ap()
    ones_d = nc.dram_tensor("ones", [128, 1], F32, kind="ExternalInput").ap()
    e40_d = nc.dram_tensor("e40", [P, RT], F32, kind="ExternalInput").ap()
    y_d = nc.dram_tensor("y", [S, OH], F32, kind="ExternalOutput").ap()

    UNFUSED = 8  # groups drained before the gate is ready

    with tile.TileContext(nc) as tc, ExitStack() as ctx:
        consts = ctx.enter_context(tc.tile_pool(name="consts", bufs=1))
        rdp = ctx.enter_context(tc.tile_pool(name="rdp", bufs=1))
        wp = ctx.enter_context(tc.tile_pool(name="wp", bufs=1))
        xp = ctx.enter_context(tc.tile_pool(name="xp", bufs=NC))
        zp = ctx.enter_context(tc.tile_pool(name="zp", bufs=1))
        sfx = ctx.enter_context(tc.tile_pool(name="sfx", bufs=1))
        yb = ctx.enter_context(tc.tile_pool(name="yb", bufs=UNFUSED // 2 + 4))
        za_ps = ctx.enter_context(tc.tile_pool(name="za_ps", bufs=2, space="PSUM"))
        y_ps = ctx.enter_context(tc.tile_pool(name="y_ps", bufs=4, space="PSUM"))
        tr_ps = ctx.enter_context(tc.tile_pool(name="tr_ps", bufs=1, space="PSUM"))
        sm_ps = ctx.enter_context(tc.tile_pool(name="sm_ps", bufs=1, space="PSUM"))

        # rd + first x chunk first (critical path to the first matmuls)
        rd_sb = rdp.tile([128, NI * ZA], F16)
        nc.sync.dma_start(rd_sb[:], rd_d)
        xt_t = {}

        def load_chunk(c):
            t = xp.tile([128, NI * 512], F16, tag="xt_t")
            nc.sync.dma_start(t[:], xt_d[:, c, :])
            xt_t[c] = t

        load_chunk(0)
        w_sb = wp.tile([128, NI * OH], F16)
        nc.sync.dma_start(w_sb[:], wt_d)
        load_chunk(1)
        eye5 = consts.tile([P, P], F32)
        ones = consts.tile([128, 1], F32)
        e40 = consts.tile([P, RT], F32)
        up_sb = consts.tile([RT, OH], F32)
        for t, d in [(eye5, eye_d), (ones, ones_d), (e40, e40_d), (up_sb, up_d)]:
            nc.sync.dma_start(t[:], d)
        for c in range(2, NC):
            load_chunk(c)

        # fused [down|route] matmul + per-chunk drain + routing transposes
        zt_sb = zp.tile([RT, S], F16)  # z^T, feeds the delta matmul
        at_sb = zp.tile([P, S], F32)  # routing logits a^T
        trp = tr_ps.tile([128, P * NG], F32)  # a, token-major, [128, 160]
        # HAM warmup: ~3.4us of junk matmuls while the first x chunk lands;
        # the transposes fully overwrite trp before the softmax reads it.
        for _ in range(32):
            nc.tensor.matmul(trp[0:ZA, 0:128], rd_sb[:, 0:ZA], rd_sb[:, 0:128], start=True, stop=True)
        for c in range(NC):
            za = za_ps.tile([ZA, 512], F32, tag="za")
            for i in range(NI):
                nc.tensor.matmul(
                    za[:],
                    rd_sb[:, ZA * i : ZA * (i + 1)],
                    xt_t[c][:, 512 * i : 512 * (i + 1)],
                    start=(i == 0),
                    stop=(i == NI - 1),
                )
            nc.vector.tensor_copy(zt_sb[:, 512 * c : 512 * (c + 1)], za[0:RT, :])
            nc.scalar.copy(at_sb[:, 512 * c : 512 * (c + 1)], za[RB : RB + P, :])
            for q in range(4):
                g = 4 * c + q
                nc.tensor.transpose(
                    trp[:, P * g : P * (g + 1)],
                    at_sb[:, 128 * g : 128 * (g + 1)],
                    eye5[:],
                )

        # softmax over experts (max-free: |a| < ~4) and token partial sums
        e_sb = sfx.tile([128, P * NG], F32)
        nc.scalar.activation(e_sb[:], trp[:], mybir.ActivationFunctionType.Exp)
        den = sfx.tile([128, NG], F32)
        nc.vector.tensor_reduce(
            den[:],
            e_sb[:].rearrange("p (g f) -> p g f", f=P),
            axis=mybir.AxisListType.X,
            op=mybir.AluOpType.add,
        )
        invd = sfx.tile([128, NG], F32)
        nc.vector.reciprocal(invd[:], den[:])
        om = sfx.tile([128, P * NG], F32)
        nc.vector.tensor_tensor(
            om[:].rearrange("p (g f) -> p g f", f=P),
            e_sb[:].rearrange("p (g f) -> p g f", f=P),
            invd[:].unsqueeze(2).to_broadcast((128, NG, P)),
            mybir.AluOpType.mult,
        )
        # token partial sums: one matmul -> [1, 160], then strided reduce
        pp = sm_ps.tile([1, P * NG], F32, tag="sm")
        nc.tensor.matmul(pp[:], ones[:, 0:1], om[:], start=True, stop=True)
        grow = sfx.tile([1, P], F32)
        nc.vector.tensor_reduce(
            grow[:],
            pp[:].rearrange("p (g f) -> p f g", f=P),
            axis=mybir.AxisListType.X,
            op=mybir.AluOpType.add,
        )
        grow = grow[:]
        # gate = 2/S * sum - 1, still as a row [1, 5]
        grow2 = sfx.tile([1, P], F32)
        nc.scalar.activation(
            grow2[:], grow, mybir.ActivationFunctionType.Copy, bias=-1.0, scale=2.0 / S
        )
        gp = sm_ps.tile([P, 1], F32, tag="sm")
        nc.tensor.transpose(gp[:], grow2[:], eye5[0:1, 0:1])
        g5 = sfx.tile([P, 1], F32)
        nc.any.tensor_copy(g5[:], gp[:])
        ep = sm_ps.tile([RT, 1], F32, tag="sm")
        nc.tensor.matmul(ep[:], e40[:], g5[:], start=True, stop=True)
        g40 = sfx.tile([RT, 1], F32)
        nc.any.tensor_copy(g40[:], ep[:])
        upeff = sfx.tile([RT, OH], F16)
        nc.vector.tensor_scalar_mul(upeff[:], up_sb[:], g40[:])

        # main y = x @ W matmuls.  The first UNFUSED groups drain their base
        # result immediately (no gate dependency); the rest take the delta
        # matmul as a 9th accumulation into the same PSUM group.
        # Groups stage in pairs so output DMAs move 512 KB each.
        y2 = {}

        def emit_group(g):
            c, q = g // 4, g % 4
            ypt = y_ps.tile([128, OH], F32, tag="ypt")
            for i in range(NI):
                nc.tensor.matmul(
                    ypt[:],
                    xt_t[c][:, 512 * i + 128 * q : 512 * i + 128 * (q + 1)],
                    w_sb[:, OH * i : OH * (i + 1)],
                    start=(i == 0),
                    stop=(i == NI - 1) and (g < UNFUSED),
                )
            if g >= UNFUSED:
                nc.tensor.matmul(
                    ypt[:],
                    zt_sb[:, 128 * g : 128 * (g + 1)],
                    upeff[:],
                    start=False,
                    stop=True,
                )
            yt = stage_slice(g)
            if g % 2 == 0:
                nc.vector.tensor_copy(yt, ypt[:])
            else:
                nc.scalar.copy(yt, ypt[:])
            if g >= UNFUSED and g % 2 == 1:
                dma_pair(g)

        def stage_slice(g):
            if g % 2 == 0:
                yt_pair = yb.tile([128, 2 * OH], F32, tag="y2")
                y2[g // 2] = yt_pair
            return y2[g // 2][:, (g % 2) * OH : (g % 2 + 1) * OH]

        def dma_pair(g):
            pair = y2[g // 2]
            dst = y_d[128 * (g - 1) : 128 * (g + 1), :].rearrange(
                "(j p) f -> p j f", j=2
            )
            nc.sync.dma_start(dst, pair[:].rearrange("p (j f) -> p j f", j=2))
        def emit_deferred_delta(g):
            dpt = y_ps.tile([128, OH], F32, tag="ypt")
            nc.tensor.matmul(
                dpt[:], zt_sb[:, 128 * g : 128 * (g + 1)], upeff[:], start=True, stop=True
            )
            ys = y2[g // 2][:, (g % 2) * OH : (g % 2 + 1) * OH]
            nc.vector.tensor_add(ys, ys, dpt[:])
            if g % 2 == 1:
                dma_pair(g)

        for g in range(UNFUSED):
            emit_group(g)
        # deferred deltas interleave between fused groups so the PE never
        # waits on the DVE adds draining their PSUM slots
        for k, g in enumerate(range(UNFUSED, NG)):
            emit_group(g)
            if k < UNFUSED:
                emit_deferred_delta(k)

    nc.compile()
    return nc


def _host_prep(inputs):
    """Shard/transform full inputs into the 8 per-core input maps."""
    x = np.asarray(inputs["input"], dtype=np.float32).reshape(B, S, IN)
    linear_w = np.asarray(inputs["linear_w"], dtype=np.float32)
    lora_down = np.asarray(inputs["lora_down"], dtype=np.float32)
    lora_up = np.asarray(inputs["lora_up"], dtype=np.float32)
    lora_route = np.asarray(inputs["lora_route"], dtype=np.float32)
    task_id = int(np.asarray(inputs["task_id"]))

    if task_id <= NUM_TASKS:
        route = lora_route[task_id - 1]  # python negative-index semantics
    else:
        route = lora_route.sum(axis=0)
    tid = min(task_id, NUM_TASKS)

    up_cat = np.zeros((RT, OUT), dtype=np.float32)
    rd = np.zeros((IN, ZA), dtype=np.float32)  # [down | pad | route]
    for p in range(tid):
        rd[:, p * R : (p + 1) * R] = lora_down[p]
        up_cat[p * R : (p + 1) * R, :] = lora_up[p]
    rd[:, RB : RB + P] = route
    rd = np.ascontiguousarray(
        rd.astype(np.float16).reshape(NI, 128, ZA).transpose(1, 0, 2).reshape(128, NI * ZA)
    )
    wt = np.ascontiguousarray(linear_w.T)  # [IN, OUT]
    eye5 = np.eye(P, dtype=np.float32)
    ones = np.ones((128, 1), dtype=np.float32)
    e40 = np.zeros((P, RT), dtype=np.float32)
    for p in range(P):
        e40[p, p * R : (p + 1) * R] = 1.0

    # x^T chunk-fused layout: [128, NC, NI*512], row p holds chunk-major data
    xts = []
    for b in range(B):
        xtb = x[b].T.astype(np.float16).reshape(NI, 128, NC, 512)
        xts.append(np.ascontiguousarray(xtb.transpose(1, 2, 0, 3).reshape(128, NC, NI * 512)))
    wts = []
    for h in range(2):
        wh = wt[:, h * OH : (h + 1) * OH].astype(np.float16).reshape(NI, 128, OH)
        wts.append(np.ascontiguousarray(wh.transpose(1, 0, 2).reshape(128, NI * OH)))
    ups = [np.ascontiguousarray(up_cat[:, h * OH : (h + 1) * OH]) for h in range(2)]

    in_maps = []
    for k in range(8):
        b, h = k // 2, k % 2
        in_maps.append(
            {
                "xt": xts[b],
                "wt": wts[h],
                "rd": rd,
                "up": ups[h],
                "eye5": eye5,
                "ones": ones,
                "e40": e40,
            }
        )
    return in_maps


def _assemble(results):
    out = np.empty((B, S, OUT), dtype=np.float32)
    for k in range(8):
        b, h = k // 2, k % 2
        out[b, :, h * OH : (h + 1) * OH] = results[k]["y"]
    return out


def kernel(**inputs) -> np.ndarray:
    nc = build_kernel()
    in_maps = _host_prep(inputs)
    res = run_bass_kernel_spmd(nc, in_maps, core_ids=list(range(8)))
    return _assemble(res.results)


if __name__ == "__main__":
    rng = np.random.default_rng(0)
    demo = {
        "input": rng.standard_normal((B, S, IN), dtype=np.float32),
        "linear_w": (rng.standard_normal((OUT, IN)) * 0.02).astype(np.float32),
        "lora_down": (rng.standard_normal((P, IN, R)) * 0.02).astype(np.float32),
        "lora_up": (rng.standard_normal((P, R, OUT)) * 0.02).astype(np.float32),
        "lora_route": (rng.standard_normal((P, IN, P)) * 0.02).astype(np.float32),
        "task_id": 5,
    }
    y = kernel(**demo)
    print("ok", y.shape, y.dtype)


# revision 23
# speedup vs baseline: 1.0396x; 1.0396x over previous
"""Trainium2 Bass kernel for ContinualLoraMoeOneGateInjectedLinear.

Computation (see reference):
    route  = lora_route[task_id-1]            (or sum over tasks if task_id > 5)
    a      = x @ route                        [B,S,P]
    gate   = 2*mean(softmax(a, -1), S) - 1    [B,P]
    tid    = min(task_id, 5)
    delta  = sum_p gate[b,p] * (x @ down_p) @ up_p   (p < tid)
    y      = x @ linear_w.T + delta

Sharding: 8 cores = 4 batches x 2 output-halves.  Core k handles batch
k//2 and output columns [512*(k%2), 512*(k%2+1)).  Each core sees all
tokens of its batch, so the gate is computed locally - no collectives.

Device kernel (per core), all heavy matmuls in float32r:
  1. zaT[45, 4096] = [down|route].T @ x   (one pass over x)
  2. transpose routing logits to [4096, 5], softmax, token-sum -> gate[5]
  3. y_base[4096, 512] = x @ Wt_half      (streams concurrently)
  4. y = y_base + zT.T @ (gate-scaled up) (added during PSUM drain)
"""

import sys

if "/opt/trn_rl_repo" not in sys.path:
    sys.path.insert(0, "/opt/trn_rl_repo")

from contextlib import ExitStack

import numpy as np

import concourse.bass as bass
import concourse.mybir as mybir
import concourse.tile as tile
from concourse import bacc
from concourse.bass_utils import run_bass_kernel_spmd

F32 = mybir.dt.float32
F32R = mybir.dt.float32r
F16 = mybir.dt.float16

NUM_TASKS = 5
B, S, IN, OUT, P, R = 4, 4096, 1024, 1024, 5, 8
RT = P * R  # 40 total low-rank dims
ZA = 72  # fused [down|route] matmul rows: 0:40 down, 64:69 route (32-aligned)
RB = 64  # partition base of the route rows
OH = OUT // 2  # 512, per-core output half
NI = IN // 128  # 8 i-tiles
NC = S // 512  # 8 token chunks of 512
NG = S // 128  # 32 token tiles of 128


def build_kernel():
    """Build the per-core Bacc module (identical program on all 8 cores)."""
    nc = bacc.Bacc("TRN2", target_bir_lowering=False, debug=False, num_devices=8)

    xt_d = nc.dram_tensor("xt", [128, NC, NI * 512], F16, kind="ExternalInput").ap()
    wt_d = nc.dram_tensor("wt", [128, NI * OH], F16, kind="ExternalInput").ap()
    rd_d = nc.dram_tensor("rd", [128, NI * ZA], F16, kind="ExternalInput").ap()
    up_d = nc.dram_tensor("up", [RT, OH], F32, kind="ExternalInput").ap()
    eye_d = nc.dram_tensor("eye5", [P, P], F32, kind="ExternalInput").ap()
    ones_d = nc.dram_tensor("ones", [128, 1], F32, kind="ExternalInput").ap()
    e40_d = nc.dram_tensor("e40", [P, RT], F32, kind="ExternalInput").ap()
    y_d = nc.dram_tensor("y", [S, OH], F32, kind="ExternalOutput").ap()

    UNFUSED = 12  # groups drained before the gate is ready

    with tile.TileContext(nc) as tc, ExitStack() as ctx:
        consts = ctx.enter_context(tc.tile_pool(name="consts", bufs=1))
        rdp = ctx.enter_context(tc.tile_pool(name="rdp", bufs=1))
        wp = ctx.enter_context(tc.tile_pool(name="wp", bufs=1))
        xp = ctx.enter_context(tc.tile_pool(name="xp", bufs=NC))
        zp = ctx.enter_context(tc.tile_pool(name="zp", bufs=1))
        sfx = ctx.enter_context(tc.tile_pool(name="sfx", bufs=1))
        yb = ctx.enter_context(tc.tile_pool(name="yb", bufs=UNFUSED // 2 + 4))
        za_ps = ctx.enter_context(tc.tile_pool(name="za_ps", bufs=2, space="PSUM"))
        y_ps = ctx.enter_context(tc.tile_pool(name="y_ps", bufs=4, space="PSUM"))
        tr_ps = ctx.enter_context(tc.tile_pool(name="tr_ps", bufs=1, space="PSUM"))
        sm_ps = ctx.enter_context(tc.tile_pool(name="sm_ps", bufs=1, space="PSUM"))

        # rd + first x chunk first (critical path to the first matmuls)
        rd_sb = rdp.tile([128, NI * ZA], F16)
        nc.sync.dma_start(rd_sb[:], rd_d)
        xt_t = {}

        def load_chunk(c):
            t = xp.tile([128, NI * 512], F16, tag="xt_t")
            nc.sync.dma_start(t[:], xt_d[:, c, :])
            xt_t[c] = t

        load_chunk(0)
        w_sb = wp.tile([128, NI * OH], F16)
        nc.sync.dma_start(w_sb[:], wt_d)
        load_chunk(1)
        eye5 = consts.tile([P, P], F32)
        ones = consts.tile([128, 1], F32)
        e40 = consts.tile([P, RT], F32)
        up_sb = consts.tile([RT, OH], F32)
        for t, d in [(eye5, eye_d), (ones, ones_d), (e40, e40_d), (up_sb, up_d)]:
            nc.sync.dma_start(t[:], d)
        for c in range(2, NC):
            load_chunk(c)

        # fused [down|route] matmul + per-chunk drain + routing transposes
        zt_sb = zp.tile([RT, S], F16)  # z^T, feeds the delta matmul
        at_sb = zp.tile([P, S], F32)  # routing logits a^T
        trp = tr_ps.tile([128, P * NG], F32)  # a, token-major, [128, 160]
        # HAM warmup: ~3.4us of junk matmuls while the first x chunk lands;
        # the transposes fully overwrite trp before the softmax reads it.
        for _ in range(32):
            nc.tensor.matmul(trp[0:ZA, 0:128], rd_sb[:, 0:ZA], rd_sb[:, 0:128], start=True, stop=True)
        for c in range(NC):
            za = za_ps.tile([ZA, 512], F32, tag="za")
            for i in range(NI):
                nc.tensor.matmul(
                    za[:],
                    rd_sb[:, ZA * i : ZA * (i + 1)],
                    xt_t[c][:, 512 * i : 512 * (i + 1)],
                    start=(i == 0),
                    stop=(i == NI - 1),
                )
            nc.vector.tensor_copy(zt_sb[:, 512 * c : 512 * (c + 1)], za[0:RT, :])
            nc.scalar.copy(at_sb[:, 512 * c : 512 * (c + 1)], za[RB : RB + P, :])
            for q in range(4):
                g = 4 * c + q
                nc.tensor.transpose(
                    trp[:, P * g : P * (g + 1)],
                    at_sb[:, 128 * g : 128 * (g + 1)],
                    eye5[:],
                )

        # softmax over experts (max-free: |a| < ~4) and token partial sums
        e_sb = sfx.tile([128, P * NG], F32)
        nc.scalar.activation(e_sb[:], trp[:], mybir.ActivationFunctionType.Exp)
        den = sfx.tile([128, NG], F32)
        nc.vector.tensor_reduce(
            den[:],
            e_sb[:].rearrange("p (g f) -> p g f", f=P),
            axis=mybir.AxisListType.X,
            op=mybir.AluOpType.add,
        )
        invd = sfx.tile([128, NG], F32)
        nc.vector.reciprocal(invd[:], den[:])
        om = sfx.tile([128, P * NG], F32)
        nc.vector.tensor_tensor(
            om[:].rearrange("p (g f) -> p g f", f=P),
            e_sb[:].rearrange("p (g f) -> p g f", f=P),
            invd[:].unsqueeze(2).to_broadcast((128, NG, P)),
            mybir.AluOpType.mult,
        )
        # token partial sums: one matmul -> [1, 160], then strided reduce
        pp = sm_ps.tile([1, P * NG], F32, tag="sm")
        nc.tensor.matmul(pp[:], ones[:, 0:1], om[:], start=True, stop=True)
        grow = sfx.tile([1, P], F32)
        nc.vector.tensor_reduce(
            grow[:],
            pp[:].rearrange("p (g f) -> p f g", f=P),
            axis=mybir.AxisListType.X,
            op=mybir.AluOpType.add,
        )
        grow = grow[:]
        # gate = 2/S * sum - 1, still as a row [1, 5]
        grow2 = sfx.tile([1, P], F32)
        nc.scalar.activation(
            grow2[:], grow, mybir.ActivationFunctionType.Copy, bias=-1.0, scale=2.0 / S
        )
        gp = sm_ps.tile([P, 1], F32, tag="sm")
        nc.tensor.transpose(gp[:], grow2[:], eye5[0:1, 0:1])
        g5 = sfx.tile([P, 1], F32)
        nc.any.tensor_copy(g5[:], gp[:])
        ep = sm_ps.tile([RT, 1], F32, tag="sm")
        nc.tensor.matmul(ep[:], e40[:], g5[:], start=True, stop=True)
        g40 = sfx.tile([RT, 1], F32)
        nc.any.tensor_copy(g40[:], ep[:])
        upeff = sfx.tile([RT, OH], F16)
        nc.vector.tensor_scalar_mul(upeff[:], up_sb[:], g40[:])

        # main y = x @ W matmuls.  The first UNFUSED groups drain their base
        # result immediately (no gate dependency); the rest take the delta
        # matmul as a 9th accumulation into the same PSUM group.
        # Groups stage in pairs so output DMAs move 512 KB each.
        y2 = {}

        def emit_group(g):
            c, q = g // 4, g % 4
            ypt = y_ps.tile([128, OH], F32, tag="ypt")
            for i in range(NI):
                nc.tensor.matmul(
                    ypt[:],
                    xt_t[c][:, 512 * i + 128 * q : 512 * i + 128 * (q + 1)],
                    w_sb[:, OH * i : OH * (i + 1)],
                    start=(i == 0),
                    stop=(i == NI - 1) and (g < UNFUSED),
                )
            if g >= UNFUSED:
                nc.tensor.matmul(
                    ypt[:],
                    zt_sb[:, 128 * g : 128 * (g + 1)],
                    upeff[:],
                    start=False,
                    stop=True,
                )
            yt = stage_slice(g)
            if g % 2 == 0:
                nc.vector.tensor_copy(yt, ypt[:])
            else:
                nc.scalar.copy(yt, ypt[:])
            if g >= UNFUSED and g % 2 == 1:
                dma_pair(g)

        def stage_slice(g):
            if g % 2 == 0:
                yt_pair = yb.tile([128, 2 * OH], F32, tag="y2")
                y2[g // 2] = yt_pair
            return y2[g // 2][:, (g % 2) * OH : (g % 2 + 1) * OH]

        def dma_pair(g):
            pair = y2[g // 2]
            dst = y_d[128 * (g - 1) : 128 * (g + 1), :].rearrange(
                "(j p) f -> p j f", j=2
            )
            nc.sync.dma_start(dst, pair[:].rearrange("p (j f) -> p j f", j=2))
        def emit_deferred_delta(g):
            dpt = y_ps.tile([128, OH], F32, tag="ypt")
            nc.tensor.matmul(
                dpt[:], zt_sb[:, 128 * g : 128 * (g + 1)], upeff[:], start=True, stop=True
            )
            ys = y2[g // 2][:, (g % 2) * OH : (g % 2 + 1) * OH]
            nc.vector.tensor_add(ys, ys, dpt[:])
            if g % 2 == 1:
                dma_pair(g)

        for g in range(UNFUSED):
            emit_group(g)
        # deferred deltas interleave between fused groups so the PE never
        # waits on the DVE adds draining their PSUM slots
        for k, g in enumerate(range(UNFUSED, NG)):
            emit_group(g)
            if k < UNFUSED:
                emit_deferred_delta(k)

    nc.compile()
    return nc


def _host_prep(inputs):
    """Shard/transform full inputs into the 8 per-core input maps."""
    x = np.asarray(inputs["input"], dtype=np.float32).reshape(B, S, IN)
    linear_w = np.asarray(inputs["linear_w"], dtype=np.float32)
    lora_down = np.asarray(inputs["lora_down"], dtype=np.float32)
    lora_up = np.asarray(inputs["lora_up"], dtype=np.float32)
    lora_route = np.asarray(inputs["lora_route"], dtype=np.float32)
    task_id = int(np.asarray(inputs["task_id"]))

    if task_id <= NUM_TASKS:
        route = lora_route[task_id - 1]  # python negative-index semantics
    else:
        route = lora_route.sum(axis=0)
    tid = min(task_id, NUM_TASKS)

    up_cat = np.zeros((RT, OUT), dtype=np.float32)
    rd = np.zeros((IN, ZA), dtype=np.float32)  # [down | pad | route]
    for p in range(tid):
        rd[:, p * R : (p + 1) * R] = lora_down[p]
        up_cat[p * R : (p + 1) * R, :] = lora_up[p]
    rd[:, RB : RB + P] = route
    rd = np.ascontiguousarray(
        rd.astype(np.float16).reshape(NI, 128, ZA).transpose(1, 0, 2).reshape(128, NI * ZA)
    )
    wt = np.ascontiguousarray(linear_w.T)  # [IN, OUT]
    eye5 = np.eye(P, dtype=np.float32)
    ones = np.ones((128, 1), dtype=np.float32)
    e40 = np.zeros((P, RT), dtype=np.float32)
    for p in range(P):
        e40[p, p * R : (p + 1) * R] = 1.0

    # x^T chunk-fused layout: [128, NC, NI*512], row p holds chunk-major data
    xts = []
    for b in range(B):
        xtb = x[b].T.astype(np.float16).reshape(NI, 128, NC, 512)
        xts.append(np.ascontiguousarray(xtb.transpose(1, 2, 0, 3).reshape(128, NC, NI * 512)))
    wts = []
    for h in range(2):
        wh = wt[:, h * OH : (h + 1) * OH].astype(np.float16).reshape(NI, 128, OH)
        wts.append(np.ascontiguousarray(wh.transpose(1, 0, 2).reshape(128, NI * OH)))
    ups = [np.ascontiguousarray(up_cat[:, h * OH : (h + 1) * OH]) for h in range(2)]

    in_maps = []
    for k in range(8):
        b, h = k // 2, k % 2
        in_maps.append(
            {
                "xt": xts[b],
                "wt": wts[h],
                "rd": rd,
                "up": ups[h],
                "eye5": eye5,
                "ones": ones,
                "e40": e40,
            }
        )
    return in_maps


def _assemble(results):
    out = np.empty((B, S, OUT), dtype=np.float32)
    for k in range(8):
        b, h = k // 2, k % 2
        out[b, :, h * OH : (h + 1) * OH] = results[k]["y"]
    return out


def kernel(**inputs) -> np.ndarray:
    nc = build_kernel()
    in_maps = _host_prep(inputs)
    res = run_bass_kernel_spmd(nc, in_maps, core_ids=list(range(8)))
    return _assemble(res.results)


if __name__ == "__main__":
    rng = np.random.default_rng(0)
    demo = {
        "input": rng.standard_normal((B, S, IN), dtype=np.float32),
        "linear_w": (rng.standard_normal((OUT, IN)) * 0.02).astype(np.float32),
        "lora_down": (rng.standard_normal((P, IN, R)) * 0.02).astype(np.float32),
        "lora_up": (rng.standard_normal((P, R, OUT)) * 0.02).astype(np.float32),
        "lora_route": (rng.standard_normal((P, IN, P)) * 0.02).astype(np.float32),
        "task_id": 5,
    }
    y = kernel(**demo)
    print("ok", y.shape, y.dtype)


# revision 24
# speedup vs baseline: 1.0537x; 1.0135x over previous
"""Trainium2 Bass kernel for ContinualLoraMoeOneGateInjectedLinear.

Computation (see reference):
    route  = lora_route[task_id-1]            (or sum over tasks if task_id > 5)
    a      = x @ route                        [B,S,P]
    gate   = 2*mean(softmax(a, -1), S) - 1    [B,P]
    tid    = min(task_id, 5)
    delta  = sum_p gate[b,p] * (x @ down_p) @ up_p   (p < tid)
    y      = x @ linear_w.T + delta

Sharding: 8 cores = 4 batches x 2 output-halves.  Core k handles batch
k//2 and output columns [512*(k%2), 512*(k%2+1)).  Each core sees all
tokens of its batch, so the gate is computed locally - no collectives.

Device kernel (per core), all heavy matmuls in float32r:
  1. zaT[45, 4096] = [down|route].T @ x   (one pass over x)
  2. transpose routing logits to [4096, 5], softmax, token-sum -> gate[5]
  3. y_base[4096, 512] = x @ Wt_half      (streams concurrently)
  4. y = y_base + zT.T @ (gate-scaled up) (added during PSUM drain)
"""

import sys

if "/opt/trn_rl_repo" not in sys.path:
    sys.path.insert(0, "/opt/trn_rl_repo")

from contextlib import ExitStack

import numpy as np

import concourse.bass as bass
import concourse.mybir as mybir
import concourse.tile as tile
from concourse import bacc
from concourse.bass_utils import run_bass_kernel_spmd

F32 = mybir.dt.float32
F32R = mybir.dt.float32r
F16 = mybir.dt.float16

NUM_TASKS = 5
B, S, IN, OUT, P, R = 4, 4096, 1024, 1024, 5, 8
RT = P * R  # 40 total low-rank dims
ZA = 72  # fused [down|route] matmul rows: 0:40 down, 64:69 route (32-aligned)
RB = 64  # partition base of the route rows
OH = OUT // 2  # 512, per-core output half
NI = IN // 128  # 8 i-tiles
NC = S // 512  # 8 token chunks of 512
NG = S // 128  # 32 token tiles of 128


def build_kernel():
    """Build the per-core Bacc module (identical program on all 8 cores)."""
    nc = bacc.Bacc("TRN2", target_bir_lowering=False, debug=False, num_devices=8)

    xt_d = nc.dram_tensor("xt", [128, NC, NI * 512], F16, kind="ExternalInput").ap()
    wt_d = nc.dram_tensor("wt", [128, NI * OH], F16, kind="ExternalInput").ap()
    rd_d = nc.dram_tensor("rd", [128, NI * ZA], F16, kind="ExternalInput").ap()
    up_d = nc.dram_tensor("up", [RT, OH], F32, kind="ExternalInput").ap()
    eye_d = nc.dram_tensor("eye5", [P, P], F32, kind="ExternalInput").ap()
    ones_d = nc.dram_tensor("ones", [128, 1], F32, kind="ExternalInput").ap()
    e40_d = nc.dram_tensor("e40", [P, RT], F32, kind="ExternalInput").ap()
    y_d = nc.dram_tensor("y", [S, OH], F32, kind="ExternalOutput").ap()

    UNFUSED = 12  # groups drained before the gate is ready

    with tile.TileContext(nc) as tc, ExitStack() as ctx:
        consts = ctx.enter_context(tc.tile_pool(name="consts", bufs=1))
        rdp = ctx.enter_context(tc.tile_pool(name="rdp", bufs=1))
        wp = ctx.enter_context(tc.tile_pool(name="wp", bufs=1))
        xp = ctx.enter_context(tc.tile_pool(name="xp", bufs=NC))
        zp = ctx.enter_context(tc.tile_pool(name="zp", bufs=1))
        sfx = ctx.enter_context(tc.tile_pool(name="sfx", bufs=1))
        yb = ctx.enter_context(tc.tile_pool(name="yb", bufs=UNFUSED // 2 + 4))
        za_ps = ctx.enter_context(tc.tile_pool(name="za_ps", bufs=2, space="PSUM"))
        y_ps = ctx.enter_context(tc.tile_pool(name="y_ps", bufs=4, space="PSUM"))
        tr_ps = ctx.enter_context(tc.tile_pool(name="tr_ps", bufs=1, space="PSUM"))
        sm_ps = ctx.enter_context(tc.tile_pool(name="sm_ps", bufs=1, space="PSUM"))

        # rd + first x chunk first (critical path to the first matmuls)
        rd_sb = rdp.tile([128, NI * ZA], F16)
        nc.sync.dma_start(rd_sb[:], rd_d)
        xt_t = {}

        def load_chunk(c):
            t = xp.tile([128, NI * 512], F16, tag="xt_t")
            nc.sync.dma_start(t[:], xt_d[:, c, :])
            xt_t[c] = t

        load_chunk(0)
        w_sb = wp.tile([128, NI * OH], F16)
        nc.sync.dma_start(w_sb[:], wt_d)
        load_chunk(1)
        eye5 = consts.tile([P, P], F32)
        ones = consts.tile([128, 1], F32)
        e40 = consts.tile([P, RT], F32)
        up_sb = consts.tile([RT, OH], F32)
        for t, d in [(eye5, eye_d), (ones, ones_d), (e40, e40_d), (up_sb, up_d)]:
            nc.sync.dma_start(t[:], d)
        for c in range(2, NC):
            load_chunk(c)

        # fused [down|route] matmul + per-chunk drain + routing transposes
        zt_sb = zp.tile([RT, S], F16)  # z^T, feeds the delta matmul
        at_sb = zp.tile([P, S], F32)  # routing logits a^T
        trp = tr_ps.tile([128, P * NG], F32)  # a, token-major, [128, 160]
        # HAM warmup: ~3.4us of junk matmuls while the first x chunk lands;
        # the transposes fully overwrite trp before the softmax reads it.
        for _ in range(32):
            nc.tensor.matmul(trp[0:ZA, 0:128], rd_sb[:, 0:ZA], rd_sb[:, 0:128], start=True, stop=True)
        for c in range(NC):
            za = za_ps.tile([ZA, 512], F32, tag="za")
            for i in range(NI):
                nc.tensor.matmul(
                    za[:],
                    rd_sb[:, ZA * i : ZA * (i + 1)],
                    xt_t[c][:, 512 * i : 512 * (i + 1)],
                    start=(i == 0),
                    stop=(i == NI - 1),
                )
            nc.vector.tensor_copy(at_sb[:, 512 * c : 512 * (c + 1)], za[RB : RB + P, :])
            nc.scalar.copy(zt_sb[:, 512 * c : 512 * (c + 1)], za[0:RT, :])
            for q in range(4):
                g = 4 * c + q
                nc.tensor.transpose(
                    trp[:, P * g : P * (g + 1)],
                    at_sb[:, 128 * g : 128 * (g + 1)],
                    eye5[:],
                )

        # softmax over experts (max-free: |a| < ~4) and token partial sums
        e_sb = sfx.tile([128, P * NG], F32)
        nc.scalar.activation(e_sb[:], trp[:], mybir.ActivationFunctionType.Exp)
        den = sfx.tile([128, NG], F32)
        nc.vector.tensor_reduce(
            den[:],
            e_sb[:].rearrange("p (g f) -> p g f", f=P),
            axis=mybir.AxisListType.X,
            op=mybir.AluOpType.add,
        )
        invd = sfx.tile([128, NG], F32)
        nc.vector.reciprocal(invd[:], den[:])
        om = sfx.tile([128, P * NG], F32)
        nc.vector.tensor_tensor(
            om[:].rearrange("p (g f) -> p g f", f=P),
            e_sb[:].rearrange("p (g f) -> p g f", f=P),
            invd[:].unsqueeze(2).to_broadcast((128, NG, P)),
            mybir.AluOpType.mult,
        )
        # token partial sums: one matmul -> [1, 160], then strided reduce
        pp = sm_ps.tile([1, P * NG], F32, tag="sm")
        nc.tensor.matmul(pp[:], ones[:, 0:1], om[:], start=True, stop=True)
        grow = sfx.tile([1, P], F32)
        nc.vector.tensor_reduce(
            grow[:],
            pp[:].rearrange("p (g f) -> p f g", f=P),
            axis=mybir.AxisListType.X,
            op=mybir.AluOpType.add,
        )
        grow = grow[:]
        # gate = 2/S * sum - 1, still as a row [1, 5]
        grow2 = sfx.tile([1, P], F32)
        nc.scalar.activation(
            grow2[:], grow, mybir.ActivationFunctionType.Copy, bias=-1.0, scale=2.0 / S
        )
        gp = sm_ps.tile([P, 1], F32, tag="sm")
        nc.tensor.transpose(gp[:], grow2[:], eye5[0:1, 0:1])
        g5 = sfx.tile([P, 1], F32)
        nc.any.tensor_copy(g5[:], gp[:])
        ep = sm_ps.tile([RT, 1], F32, tag="sm")
        nc.tensor.matmul(ep[:], e40[:], g5[:], start=True, stop=True)
        g40 = sfx.tile([RT, 1], F32)
        nc.any.tensor_copy(g40[:], ep[:])
        upeff = sfx.tile([RT, OH], F16)
        nc.vector.tensor_scalar_mul(upeff[:], up_sb[:], g40[:])

        # main y = x @ W matmuls.  The first UNFUSED groups drain their base
        # result immediately (no gate dependency); the rest take the delta
        # matmul as a 9th accumulation into the same PSUM group.
        # Groups stage in pairs so output DMAs move 512 KB each.
        y2 = {}

        def emit_group(g):
            c, q = g // 4, g % 4
            ypt = y_ps.tile([128, OH], F32, tag="ypt")
            for i in range(NI):
                nc.tensor.matmul(
                    ypt[:],
                    xt_t[c][:, 512 * i + 128 * q : 512 * i + 128 * (q + 1)],
                    w_sb[:, OH * i : OH * (i + 1)],
                    start=(i == 0),
                    stop=(i == NI - 1) and (g < UNFUSED),
                )
            if g >= UNFUSED:
                nc.tensor.matmul(
                    ypt[:],
                    zt_sb[:, 128 * g : 128 * (g + 1)],
                    upeff[:],
                    start=False,
                    stop=True,
                )
            yt = stage_slice(g)
            if g % 2 == 0:
                nc.vector.tensor_copy(yt, ypt[:])
            else:
                nc.scalar.copy(yt, ypt[:])
            if g >= UNFUSED and g % 2 == 1:
                dma_pair(g)

        def stage_slice(g):
            if g % 2 == 0:
                yt_pair = yb.tile([128, 2 * OH], F32, tag="y2")
                y2[g // 2] = yt_pair
            return y2[g // 2][:, (g % 2) * OH : (g % 2 + 1) * OH]

        def dma_pair(g):
            pair = y2[g // 2]
            dst = y_d[128 * (g - 1) : 128 * (g + 1), :].rearrange(
                "(j p) f -> p j f", j=2
            )
            nc.sync.dma_start(dst, pair[:].rearrange("p (j f) -> p j f", j=2))
        def emit_deferred_delta(g):
            dpt = y_ps.tile([128, OH], F32, tag="ypt")
            nc.tensor.matmul(
                dpt[:], zt_sb[:, 128 * g : 128 * (g + 1)], upeff[:], start=True, stop=True
            )
            ys = y2[g // 2][:, (g % 2) * OH : (g % 2 + 1) * OH]
            nc.vector.tensor_add(ys, ys, dpt[:])
            if g % 2 == 1:
                dma_pair(g)

        for g in range(UNFUSED):
            emit_group(g)
        # deferred deltas interleave between fused groups so the PE never
        # waits on the DVE adds draining their PSUM slots
        for k, g in enumerate(range(UNFUSED, NG)):
            emit_group(g)
            if k < UNFUSED:
                emit_deferred_delta(k)

    nc.compile()
    return nc


def _host_prep(inputs):
    """Shard/transform full inputs into the 8 per-core input maps."""
    x = np.asarray(inputs["input"], dtype=np.float32).reshape(B, S, IN)
    linear_w = np.asarray(inputs["linear_w"], dtype=np.float32)
    lora_down = np.asarray(inputs["lora_down"], dtype=np.float32)
    lora_up = np.asarray(inputs["lora_up"], dtype=np.float32)
    lora_route = np.asarray(inputs["lora_route"], dtype=np.float32)
    task_id = int(np.asarray(inputs["task_id"]))

    if task_id <= NUM_TASKS:
        route = lora_route[task_id - 1]  # python negative-index semantics
    else:
        route = lora_route.sum(axis=0)
    tid = min(task_id, NUM_TASKS)

    up_cat = np.zeros((RT, OUT), dtype=np.float32)
    rd = np.zeros((IN, ZA), dtype=np.float32)  # [down | pad | route]
    for p in range(tid):
        rd[:, p * R : (p + 1) * R] = lora_down[p]
        up_cat[p * R : (p + 1) * R, :] = lora_up[p]
    rd[:, RB : RB + P] = route
    rd = np.ascontiguousarray(
        rd.astype(np.float16).reshape(NI, 128, ZA).transpose(1, 0, 2).reshape(128, NI * ZA)
    )
    wt = np.ascontiguousarray(linear_w.T)  # [IN, OUT]
    eye5 = np.eye(P, dtype=np.float32)
    ones = np.ones((128, 1), dtype=np.float32)
    e40 = np.zeros((P, RT), dtype=np.float32)
    for p in range(P):
        e40[p, p * R : (p + 1) * R] = 1.0

    # x^T chunk-fused layout: [128, NC, NI*512], row p holds chunk-major data
    xts = []
    for b in range(B):
        xtb = x[b].T.astype(np.float16).reshape(NI, 128, NC, 512)
        xts.append(np.ascontiguousarray(xtb.transpose(1, 2, 0, 3).reshape(128, NC, NI * 512)))
    wts = []
    for h in range(2):
        wh = wt[:, h * OH : (h + 1) * OH].astype(np.float16).reshape(NI, 128, OH)
        wts.append(np.ascontiguousarray(wh.transpose(1, 0, 2).reshape(128, NI * OH)))
    ups = [np.ascontiguousarray(up_cat[:, h * OH : (h + 1) * OH]) for h in range(2)]

    in_maps = []
    for k in range(8):
        b, h = k // 2, k % 2
        in_maps.append(
            {
                "xt": xts[b],
                "wt": wts[h],
                "rd": rd,
                "up": ups[h],
                "eye5": eye5,
                "ones": ones,
                "e40": e40,
            }
        )
    return in_maps


def _assemble(results):
    out = np.empty((B, S, OUT), dtype=np.float32)
    for k in range(8):
        b, h = k // 2, k % 2
        out[b, :, h * OH : (h + 1) * OH] = results[k]["y"]
    return out


def kernel(**inputs) -> np.ndarray:
    nc = build_kernel()
    in_maps = _host_prep(inputs)
    res = run_bass_kernel_spmd(nc, in_maps, core_ids=list(range(8)))
    return _assemble(res.results)


if __name__ == "__main__":
    rng = np.random.default_rng(0)
    demo = {
        "input": rng.standard_normal((B, S, IN), dtype=np.float32),
        "linear_w": (rng.standard_normal((OUT, IN)) * 0.02).astype(np.float32),
        "lora_down": (rng.standard_normal((P, IN, R)) * 0.02).astype(np.float32),
        "lora_up": (rng.standard_normal((P, R, OUT)) * 0.02).astype(np.float32),
        "lora_route": (rng.standard_normal((P, IN, P)) * 0.02).astype(np.float32),
        "task_id": 5,
    }
    y = kernel(**demo)
    print("ok", y.shape, y.dtype)
